# revision 30
# baseline (speedup 1.0000x reference)
"""Trainium2 Bass kernel for ApertureChamberSSM (v5, chunk-blocked even/odd).

Computation (reference):
    iv, ov, beta_s, alpha, mg = sigmoid(scalars); decay = exp(-alpha)
    x_in  = iv * x ; drive = tanh(x_in)
    psi_s = decay * psi_{s-1} + (1-decay) * drive_s          (scan over S)
    x_mem = mg * psi + (1-mg) * x_in
    rotate channel pairs (j, j+512) by pi*sigmoid(beta), scale by ov

Algebra: psi = (1-decay)*psi' with psi'_s = decay*psi'_{s-1} + drive_s
    x_mem = ap_*psi' + c*x   (ap_ = mg*(1-decay), c = (1-mg)*iv)
    out   = R @ x_mem = A' @ psi' + B' @ x   per channel pair, with
    R = ov*[[cos,-sin],[sin,cos]] as a dense 2x2-block 128x128 matrix.

Even/odd decimation (halves the DVE recurrence length):
    e_m         = decay*drive_{2m} + drive_{2m+1}        (DVE stt)
    po_m        = psi'_{2m+1} = decay^2*po_{m-1} + e_m   (DVE scan, half len)
    psi'_{2m}   = decay*po_{m-1} + drive_{2m}            (folded into PE)
    out_odd  = A @ po + B @ x_odd                        (2 matmuls/sub)
    out_even = (decay*A) @ po_shift + A @ drive_even + B @ x_even   (3)

Layout: each SBUF tile holds 64 real channels (partitions 0..63) and their
paired imag channels (64..127). The host stores each row's sequence
chunk-blocked: chunk k occupies columns [2048k, 2048(k+1)) as
[1024 even positions | 1024 odd positions], so every chunk needs exactly
one input DMA, one tanh, one eviction, one output DMA of [128, 2048].
Per core: 4 row-tiles (one per batch) x 4 chunks. 8 cores, zero comms.
"""

import math

import numpy as np

B, S, D = 4, 8192, 1024
HALF = D // 2          # 512
NCORES = 8
JPC = HALF // NCORES   # 64 channel pairs per core
ROWS = 2 * B * JPC     # 512 rows per core
P = 128                # partitions
C = 2048               # seq positions per chunk (1024 even + 1024 odd)
CP = C // 2            # 1024 pairs per chunk
NCHUNK = S // C        # 4
NTILE = ROWS // P      # 4 row-tiles per core (one per batch)
MMF = 512              # matmul moving free dim (one PSUM bank)

_cache = {}


def _sig(v):
    return 1.0 / (1.0 + math.exp(-float(v)))


def _build(iv, decay, use_scan):
    import concourse.bass as bass
    import concourse.tile as tile
    from concourse import bacc, mybir

    f32 = mybir.dt.float32
    bf16 = mybir.dt.bfloat16
    AF = mybir.ActivationFunctionType
    OP = mybir.AluOpType

    nc = bacc.Bacc("TRN2", target_bir_lowering=False, debug=False,
                   num_devices=NCORES)
    x_ap = nc.dram_tensor("x", [ROWS, S], bf16, kind="ExternalInput").ap()
    consts_ap = nc.dram_tensor("consts", [P, 3 * P], bf16,
                               kind="ExternalInput").ap()
    out_ap = nc.dram_tensor("out", [ROWS, S], bf16, kind="ExternalOutput").ap()

    with tile.TileContext(nc) as tc:
        with (
            tc.tile_pool(name="const", bufs=1) as cpool,
            tc.tile_pool(name="xin", bufs=3) as xpool,
            tc.tile_pool(name="drv", bufs=3) as dpool,
            tc.tile_pool(name="ebuf", bufs=3) as epool,
            tc.tile_pool(name="psi", bufs=3) as ppool,
            tc.tile_pool(name="outs", bufs=3) as opool,
            tc.tile_pool(name="ps", bufs=1, space=bass.MemorySpace.PSUM) as pspool,
        ):
            idm = cpool.tile([P, 3 * P], bf16, tag="idm")
            nc.sync.dma_start(idm[:], consts_ap[:])
            A_m = idm[:, 0:P]           # (ap_*R)^T
            B_m = idm[:, P:2 * P]       # (c*R)^T
            dA_m = idm[:, 2 * P:3 * P]  # (decay*ap_*R)^T

            if use_scan:
                dk2 = cpool.tile([P, CP], f32, tag="dk2")
                nc.vector.memset(dk2[:], decay * decay)

            prev = [None] * NTILE

            def front(k, b):
                r0 = b * P
                x_t = xpool.tile([P, C], bf16, tag=f"x{b}")
                if k == 0:
                    # split the cold-start chunk so the first tanh/scan can
                    # begin as soon as the first half-DMA lands
                    nc.sync.dma_start(x_t[:, 0:CP], x_ap[r0:r0 + P, 0:CP])
                    nc.sync.dma_start(x_t[:, CP:C], x_ap[r0:r0 + P, CP:C])
                else:
                    nc.sync.dma_start(
                        x_t[:], x_ap[r0:r0 + P, k * C:(k + 1) * C])
                d_t = p_t = None
                if use_scan:
                    d_t = dpool.tile([P, C], bf16, tag=f"d{b}")
                    if k == 0:
                        nc.scalar.activation(d_t[:, 0:CP], x_t[:, 0:CP],
                                             AF.Tanh, bias=0.0, scale=iv)
                        nc.scalar.activation(d_t[:, CP:C], x_t[:, CP:C],
                                             AF.Tanh, bias=0.0, scale=iv)
                    else:
                        nc.scalar.activation(d_t[:], x_t[:], AF.Tanh,
                                             bias=0.0, scale=iv)
                    e_t = epool.tile([P, CP], bf16, tag=f"e{b}")
                    nc.vector.scalar_tensor_tensor(
                        e_t[:], d_t[:, 0:CP], decay, d_t[:, CP:C],
                        OP.mult, OP.add)
                    p_t = ppool.tile([P, CP + 1], bf16, tag=f"p{b}")
                    if prev[b] is None:
                        nc.vector.memset(p_t[:, 0:1], 0.0)
                        init = 0.0
                    else:
                        nc.vector.tensor_copy(p_t[:, 0:1], prev[b])
                        init = prev[b]
                    nc.vector.tensor_tensor_scan(
                        p_t[:, 1:CP + 1], dk2[:], e_t[:], init,
                        OP.mult, OP.add)
                    prev[b] = p_t[:, CP:CP + 1]
                return x_t, d_t, p_t

            def back(k, b, x_t, d_t, p_t):
                r0 = b * P
                o_t = opool.tile([P, C], bf16, tag=f"o{b}")
                ps = pspool.tile([P, C], f32, tag=f"ps{b % 2}")
                for s4 in range(CP // MMF):
                    fs = slice(s4 * MMF, (s4 + 1) * MMF)      # even cols
                    fo = slice(CP + s4 * MMF, CP + (s4 + 1) * MMF)  # odd
                    fp = slice(1 + s4 * MMF, 1 + (s4 + 1) * MMF)    # po
                    fps = slice(s4 * MMF, (s4 + 1) * MMF)     # po shifted
                    if use_scan:
                        # evens: dA @ po_shift + A @ drive_even + B @ x_even
                        nc.tensor.matmul(ps[:, fs], dA_m, p_t[:, fps],
                                         start=True, stop=False)
                        nc.tensor.matmul(ps[:, fs], A_m, d_t[:, fs],
                                         start=False, stop=False)
                        nc.tensor.matmul(ps[:, fs], B_m, x_t[:, fs],
                                         start=False, stop=True)
                        # odds: A @ po + B @ x_odd
                        nc.tensor.matmul(ps[:, fo], A_m, p_t[:, fp],
                                         start=True, stop=False)
                        nc.tensor.matmul(ps[:, fo], B_m, x_t[:, fo],
                                         start=False, stop=True)
                    else:
                        nc.tensor.matmul(ps[:, fs], B_m, x_t[:, fs],
                                         start=True, stop=True)
                        nc.tensor.matmul(ps[:, fo], B_m, x_t[:, fo],
                                         start=True, stop=True)
                nc.scalar.copy(o_t[:], ps[:])
                nc.sync.dma_start(
                    out_ap[r0:r0 + P, k * C:(k + 1) * C], o_t[:])

            pend = None
            for k in range(NCHUNK):
                for b in range(NTILE):
                    cur = front(k, b)
                    if pend is not None:
                        back(*pend)
                    pend = (k, b, *cur)
            back(*pend)

    nc.compile()
    return nc


def kernel(x, beta, input_valve, output_valve, alpha_raw, memory_gate):
    x = np.asarray(x, dtype=np.float32)
    assert x.shape == (B, S, D), x.shape

    beta_s = _sig(beta)
    iv = _sig(input_valve)
    ov = _sig(output_valve)
    alpha = _sig(alpha_raw)
    mg = _sig(memory_gate)
    decay = math.exp(-alpha)
    c = (1.0 - mg) * iv
    ap_ = mg * (1.0 - decay)
    angle = math.pi * beta_s
    p_, q_ = math.cos(angle) * ov, math.sin(angle) * ov
    use_scan = ap_ != 0.0

    key = (round(iv, 12), round(decay, 12), use_scan)
    if key not in _cache:
        _cache[key] = _build(iv, decay, use_scan)
    nc = _cache[key]

    import ml_dtypes
    from concourse.bass_utils import run_bass_kernel_spmd

    bf = ml_dtypes.bfloat16
    h = P // 2
    eye = np.eye(h, dtype=np.float64)
    Rt = np.zeros((P, P))
    Rt[:h, :h] = p_ * eye
    Rt[:h, h:] = q_ * eye
    Rt[h:, :h] = -q_ * eye
    Rt[h:, h:] = p_ * eye
    consts = np.concatenate(
        [ap_ * Rt, c * Rt, decay * ap_ * Rt], axis=1).astype(bf)

    # shard rows: tile b: 64 real then 64 imag channels; columns: sequence
    # chunk-blocked, chunk k = [1024 evens | 1024 odds] of positions
    # [2048k, 2048(k+1))
    xr = x[:, :, :HALF].reshape(B, S, NCORES, JPC)
    xi = x[:, :, HALF:].reshape(B, S, NCORES, JPC)
    in_maps = []
    for cix in range(NCORES):
        shard = np.empty((NTILE, 2, JPC, NCHUNK, 2, CP), dtype=bf)
        for b in range(B):
            # (JPC, S) -> (JPC, NCHUNK, CP, 2) -> eo-blocked
            r = xr[b, :, cix, :].T.astype(bf).reshape(JPC, NCHUNK, CP, 2)
            i_ = xi[b, :, cix, :].T.astype(bf).reshape(JPC, NCHUNK, CP, 2)
            shard[b, 0] = r.transpose(0, 1, 3, 2)
            shard[b, 1] = i_.transpose(0, 1, 3, 2)
        in_maps.append({"x": shard.reshape(ROWS, S), "consts": consts})

    res = run_bass_kernel_spmd(nc, in_maps, core_ids=list(range(NCORES)))
    global last_result
    last_result = res

    out = np.empty((B, S, D), dtype=np.float32)
    o_r = out[:, :, :HALF].reshape(B, S, NCORES, JPC)
    o_i = out[:, :, HALF:].reshape(B, S, NCORES, JPC)
    for cix in range(NCORES):
        oc = np.asarray(res.results[cix]["out"]).reshape(
            NTILE, 2, JPC, NCHUNK, 2, CP)
        for b in range(B):
            rr = oc[b, 0].transpose(0, 1, 3, 2).reshape(JPC, S)
            ii = oc[b, 1].transpose(0, 1, 3, 2).reshape(JPC, S)
            o_r[b, :, cix, :] = rr.T.astype(np.float32)
            o_i[b, :, cix, :] = ii.T.astype(np.float32)
    return out


# revision 34
# speedup vs baseline: 1.0146x; 1.0146x over previous
"""Trainium2 Bass kernel for ApertureChamberSSM (v5, chunk-blocked even/odd).

Computation (reference):
    iv, ov, beta_s, alpha, mg = sigmoid(scalars); decay = exp(-alpha)
    x_in  = iv * x ; drive = tanh(x_in)
    psi_s = decay * psi_{s-1} + (1-decay) * drive_s          (scan over S)
    x_mem = mg * psi + (1-mg) * x_in
    rotate channel pairs (j, j+512) by pi*sigmoid(beta), scale by ov

Algebra: psi = (1-decay)*psi' with psi'_s = decay*psi'_{s-1} + drive_s
    x_mem = ap_*psi' + c*x   (ap_ = mg*(1-decay), c = (1-mg)*iv)
    out   = R @ x_mem = A' @ psi' + B' @ x   per channel pair, with
    R = ov*[[cos,-sin],[sin,cos]] as a dense 2x2-block 128x128 matrix.

Even/odd decimation (halves the DVE recurrence length):
    e_m         = decay*drive_{2m} + drive_{2m+1}        (DVE stt)
    po_m        = psi'_{2m+1} = decay^2*po_{m-1} + e_m   (DVE scan, half len)
    psi'_{2m}   = decay*po_{m-1} + drive_{2m}            (folded into PE)
    out_odd  = A @ po + B @ x_odd                        (2 matmuls/sub)
    out_even = (decay*A) @ po_shift + A @ drive_even + B @ x_even   (3)

Layout: each SBUF tile holds 64 real channels (partitions 0..63) and their
paired imag channels (64..127). The host stores each row's sequence
chunk-blocked: chunk k occupies columns [2048k, 2048(k+1)) as
[1024 even positions | 1024 odd positions], so every chunk needs exactly
one input DMA, one tanh, one eviction, one output DMA of [128, 2048].
Per core: 4 row-tiles (one per batch) x 4 chunks. 8 cores, zero comms.
"""

import math

import numpy as np

B, S, D = 4, 8192, 1024
HALF = D // 2          # 512
NCORES = 8
JPC = HALF // NCORES   # 64 channel pairs per core
ROWS = 2 * B * JPC     # 512 rows per core
P = 128                # partitions
C = 2048               # seq positions per chunk (1024 even + 1024 odd)
CP = C // 2            # 1024 pairs per chunk
NCHUNK = S // C        # 4
NTILE = ROWS // P      # 4 row-tiles per core (one per batch)
MMF = 512              # matmul moving free dim (one PSUM bank)

_cache = {}


def _sig(v):
    return 1.0 / (1.0 + math.exp(-float(v)))


def _build(iv, decay, use_scan):
    import concourse.bass as bass
    import concourse.tile as tile
    from concourse import bacc, mybir

    f32 = mybir.dt.float32
    bf16 = mybir.dt.bfloat16
    AF = mybir.ActivationFunctionType
    OP = mybir.AluOpType

    nc = bacc.Bacc("TRN2", target_bir_lowering=False, debug=False,
                   num_devices=NCORES)
    x_ap = nc.dram_tensor("x", [ROWS, S], bf16, kind="ExternalInput").ap()
    consts_ap = nc.dram_tensor("consts", [P, 3 * P], bf16,
                               kind="ExternalInput").ap()
    out_ap = nc.dram_tensor("out", [ROWS, S], bf16, kind="ExternalOutput").ap()

    with tile.TileContext(nc) as tc:
        with (
            tc.tile_pool(name="const", bufs=1) as cpool,
            tc.tile_pool(name="xin", bufs=3) as xpool,
            tc.tile_pool(name="drv", bufs=3) as dpool,
            tc.tile_pool(name="ebuf", bufs=2) as epool,
            tc.tile_pool(name="psi", bufs=3) as ppool,
            tc.tile_pool(name="outs", bufs=3) as opool,
            tc.tile_pool(name="ps", bufs=1, space=bass.MemorySpace.PSUM) as pspool,
        ):
            idm = cpool.tile([P, 3 * P], bf16, tag="idm")
            nc.sync.dma_start(idm[:], consts_ap[:])
            A_m = idm[:, 0:P]           # (ap_*R)^T
            B_m = idm[:, P:2 * P]       # (c*R)^T
            dA_m = idm[:, 2 * P:3 * P]  # (decay*ap_*R)^T

            if use_scan:
                dk2 = cpool.tile([P, CP], f32, tag="dk2")
                nc.vector.memset(dk2[:], decay * decay)

            prev = [None] * NTILE

            def ebuild(d_t, e_t, tmp, lo, hi):
                # e = decay*d_even + d_odd using the fast DVE modes
                # (tensor_scalar 4x + tensor_tensor 2x beat one 1x stt)
                nc.vector.tensor_scalar_mul(
                    tmp[:, lo:hi], d_t[:, lo:hi], decay)
                nc.vector.tensor_add(
                    e_t[:, lo:hi], tmp[:, lo:hi], d_t[:, CP + lo:CP + hi])

            def front(k, b):
                r0 = b * P
                k0 = k * C
                x_t = xpool.tile([P, C], bf16, tag=f"x{b}")
                if k == 0:
                    # cold start: DMA/tanh in even-odd-paired quarters so the
                    # first half-scan starts as soon as the first pair lands
                    pieces = [(0, CP // 2), (CP, CP + CP // 2),
                              (CP // 2, CP), (CP + CP // 2, C)]
                else:
                    pieces = [(0, C)]
                for lo, hi in pieces:
                    nc.sync.dma_start(
                        x_t[:, lo:hi], x_ap[r0:r0 + P, k0 + lo:k0 + hi])
                d_t = p_t = None
                if use_scan:
                    d_t = dpool.tile([P, C], bf16, tag=f"d{b}")
                    for lo, hi in pieces:
                        nc.scalar.activation(d_t[:, lo:hi], x_t[:, lo:hi],
                                             AF.Tanh, bias=0.0, scale=iv)
                    e_t = epool.tile([P, CP], bf16, tag=f"e{b}")
                    tmp = epool.tile([P, CP], bf16, tag=f"t{b}")
                    p_t = ppool.tile([P, CP + 1], bf16, tag=f"p{b}")
                    if prev[b] is None:
                        nc.vector.memset(p_t[:, 0:1], 0.0)
                        init = 0.0
                    else:
                        nc.vector.tensor_copy(p_t[:, 0:1], prev[b])
                        init = prev[b]
                    segs = ([(0, CP // 2), (CP // 2, CP)] if k == 0
                            else [(0, CP)])
                    for lo, hi in segs:
                        ebuild(d_t, e_t, tmp, lo, hi)
                        nc.vector.tensor_tensor_scan(
                            p_t[:, 1 + lo:1 + hi], dk2[:, lo:hi],
                            e_t[:, lo:hi], init, OP.mult, OP.add)
                        init = p_t[:, hi:hi + 1]
                    prev[b] = p_t[:, CP:CP + 1]
                return x_t, d_t, p_t

            def back(k, b, x_t, d_t, p_t, last=False):
                r0 = b * P
                o_t = opool.tile([P, C], bf16, tag=f"o{b}")
                ps = pspool.tile([P, C], f32, tag=f"ps{b % 2}")
                for s4 in range(CP // MMF):
                    fs = slice(s4 * MMF, (s4 + 1) * MMF)      # even cols
                    fo = slice(CP + s4 * MMF, CP + (s4 + 1) * MMF)  # odd
                    fp = slice(1 + s4 * MMF, 1 + (s4 + 1) * MMF)    # po
                    fps = slice(s4 * MMF, (s4 + 1) * MMF)     # po shifted
                    if use_scan:
                        # evens: dA @ po_shift + A @ drive_even + B @ x_even
                        nc.tensor.matmul(ps[:, fs], dA_m, p_t[:, fps],
                                         start=True, stop=False)
                        nc.tensor.matmul(ps[:, fs], A_m, d_t[:, fs],
                                         start=False, stop=False)
                        nc.tensor.matmul(ps[:, fs], B_m, x_t[:, fs],
                                         start=False, stop=True)
                        # odds: A @ po + B @ x_odd
                        nc.tensor.matmul(ps[:, fo], A_m, p_t[:, fp],
                                         start=True, stop=False)
                        nc.tensor.matmul(ps[:, fo], B_m, x_t[:, fo],
                                         start=False, stop=True)
                    else:
                        nc.tensor.matmul(ps[:, fs], B_m, x_t[:, fs],
                                         start=True, stop=True)
                        nc.tensor.matmul(ps[:, fo], B_m, x_t[:, fo],
                                         start=True, stop=True)
                    if last:
                        # drain the epilogue at 512 granularity so the final
                        # eviction/DMA overlap the remaining matmuls
                        for seg in (fs, fo):
                            nc.scalar.copy(o_t[:, seg], ps[:, seg])
                            nc.sync.dma_start(
                                out_ap[r0:r0 + P,
                                       k * C + seg.start:k * C + seg.stop],
                                o_t[:, seg])
                if not last:
                    nc.scalar.copy(o_t[:], ps[:])
                    nc.sync.dma_start(
                        out_ap[r0:r0 + P, k * C:(k + 1) * C], o_t[:])

            pend = None
            for k in range(NCHUNK):
                for b in range(NTILE):
                    cur = front(k, b)
                    if pend is not None:
                        back(*pend)
                    pend = (k, b, *cur)
            back(*pend, last=True)

    nc.compile()
    return nc


def kernel(x, beta, input_valve, output_valve, alpha_raw, memory_gate):
    x = np.asarray(x, dtype=np.float32)
    assert x.shape == (B, S, D), x.shape

    beta_s = _sig(beta)
    iv = _sig(input_valve)
    ov = _sig(output_valve)
    alpha = _sig(alpha_raw)
    mg = _sig(memory_gate)
    decay = math.exp(-alpha)
    c = (1.0 - mg) * iv
    ap_ = mg * (1.0 - decay)
    angle = math.pi * beta_s
    p_, q_ = math.cos(angle) * ov, math.sin(angle) * ov
    use_scan = ap_ != 0.0

    key = (round(iv, 12), round(decay, 12), use_scan)
    if key not in _cache:
        _cache[key] = _build(iv, decay, use_scan)
    nc = _cache[key]

    import ml_dtypes
    from concourse.bass_utils import run_bass_kernel_spmd

    bf = ml_dtypes.bfloat16
    h = P // 2
    eye = np.eye(h, dtype=np.float64)
    Rt = np.zeros((P, P))
    Rt[:h, :h] = p_ * eye
    Rt[:h, h:] = q_ * eye
    Rt[h:, :h] = -q_ * eye
    Rt[h:, h:] = p_ * eye
    consts = np.concatenate(
        [ap_ * Rt, c * Rt, decay * ap_ * Rt], axis=1).astype(bf)

    # shard rows: tile b: 64 real then 64 imag channels; columns: sequence
    # chunk-blocked, chunk k = [1024 evens | 1024 odds] of positions
    # [2048k, 2048(k+1))
    xr = x[:, :, :HALF].reshape(B, S, NCORES, JPC)
    xi = x[:, :, HALF:].reshape(B, S, NCORES, JPC)
    in_maps = []
    for cix in range(NCORES):
        shard = np.empty((NTILE, 2, JPC, NCHUNK, 2, CP), dtype=bf)
        for b in range(B):
            # (JPC, S) -> (JPC, NCHUNK, CP, 2) -> eo-blocked
            r = xr[b, :, cix, :].T.astype(bf).reshape(JPC, NCHUNK, CP, 2)
            i_ = xi[b, :, cix, :].T.astype(bf).reshape(JPC, NCHUNK, CP, 2)
            shard[b, 0] = r.transpose(0, 1, 3, 2)
            shard[b, 1] = i_.transpose(0, 1, 3, 2)
        in_maps.append({"x": shard.reshape(ROWS, S), "consts": consts})

    res = run_bass_kernel_spmd(nc, in_maps, core_ids=list(range(NCORES)))
    global last_result
    last_result = res

    out = np.empty((B, S, D), dtype=np.float32)
    o_r = out[:, :, :HALF].reshape(B, S, NCORES, JPC)
    o_i = out[:, :, HALF:].reshape(B, S, NCORES, JPC)
    for cix in range(NCORES):
        oc = np.asarray(res.results[cix]["out"]).reshape(
            NTILE, 2, JPC, NCHUNK, 2, CP)
        for b in range(B):
            rr = oc[b, 0].transpose(0, 1, 3, 2).reshape(JPC, S)
            ii = oc[b, 1].transpose(0, 1, 3, 2).reshape(JPC, S)
            o_r[b, :, cix, :] = rr.T.astype(np.float32)
            o_i[b, :, cix, :] = ii.T.astype(np.float32)
    return out
